# revision 1
# baseline (speedup 1.0000x reference)
"""GAT-VGAE forward pass on 8 Trainium2 NeuronCores (Bass/Tile).

Dense-adjacency restructure (v2)
--------------------------------
- Edges are rasterized on the host into a dense multiplicity matrix
  A[src, dst] (counts incl. self loops).  Each core owns 256 dst nodes and
  gets the fp8 slice A_c [2048 src, 256 dst].  The GAT edge pass becomes
  dense tile math: logits = a_src[s] (+) a_dst[d], leaky-relu (one fused
  scalar_tensor_tensor), exp on ACT, multiply by A (zeros kill non-edges,
  counts weight multi-edges).  M = A*exp(leaky(.)) is the bf16 lhsT of the
  aggregation matmuls; a ones-column in the rhs yields the softmax
  denominators in the same matmul.  No dma_gather, no one-hots, no GPSIMD.
- Attention dot products are folded into the layer matmuls on the host:
  W1' = [W1 | W1@blockdiag(att_src1)]; a_dst1 for the local 256 dsts comes
  from a tiny on-device matmul W1adT @ x_localT, broadcast across
  partitions via a DMA round trip.  Layer 2 likewise ships
  W2' = [W2 | W2@att_src2 | W2@att_dst2].
- One AllGather moves the bf16 [256, 67] local table (ones|h2|a_src2|
  a_dst2); one AllReduce combines the 64-float z-sums.
- Decoder Wd is quantized to fp8 (x16, clipped to +-240, exact on TRN
  e4m3 range) and split: 62.5% of columns go through the PE as [128,128]
  lhsT tiles (rhs = packed fp8 z-mean), 37.5% are dot-producted on the
  otherwise-idle DVE (bf16 multiply + reduce against a broadcast z-mean).
  Both streams ride a deep SBUF prefetch pool filled from t=0 so the HBM
  stream overlaps all earlier phases.
"""
import sys

sys.path.insert(0, '/opt/trn_rl_repo')

import ml_dtypes
import numpy as np

import bass_rust
import concourse.bass as bass
import concourse.bacc as bacc
import concourse.mybir as mybir
import concourse.tile as tile
from concourse.bass_utils import run_bass_kernel_spmd
from concourse.masks import make_identity

F32 = mybir.dt.float32
BF16 = mybir.dt.bfloat16
F8 = mybir.dt.float8e4
AF = mybir.ActivationFunctionType
OP = mybir.AluOpType

P = 128
N = 2048
NB = 16               # 128-row source blocks
F_IN = 256
C1 = 128
H = 4
HID = 512
EMB = 64
NCORES = 8
DPC = 256             # dst nodes per core
COLS = N * N // NCORES
NEG = 0.2
AUGW = 516            # [1|h0|1|h1|1|h2|1|h3] (4*129)
H2W = 67              # [1 | h2 (64) | asrc2 | adst2]
RG = [list(range(NCORES))]

# decoder split
WD_GROUP = 32         # PE lhsT tiles per DMA group ([128, 4096] fp8)
NG_PE = 56            # PE groups of 8192 columns
PE_COLS = NG_PE * WD_GROUP * 256
PE_ROUNDS = (NG_PE + 7) // 8
NG_DVE = 8            # DVE granules of 8192 cols ([128, 64, 64] fp8)
DVE_COLS = NG_DVE * 8192
assert PE_COLS + DVE_COLS == COLS
SW = 16.0             # host scale on Wd before fp8 cast
SZ = 0.5              # on-device scale on zsum before fp8 cast
DESC_PE = 1.0 / (SW * SZ * N)
DESC_DVE = 1.0 / (SW * N)
WPE_BUFS = 15         # prefetch depth (SBUF) for PE wd stream
WDVE_BUFS = 4         # prefetch depth for DVE wd stream

_MAX_WAITS = 1
_wait_ctr = [0]


def _split_excess_waits(nc):
    """This container's walrus accepts only one sync-wait per instruction.
    Hoist excess waits onto InstNoOps inserted just before, same engine."""
    for f in nc.m.functions:
        for blk in f.blocks:
            out = []
            changed = False
            for inst in blk.instructions:
                si = inst.sync_info
                waits = list(si.on_wait) if si is not None else []
                if len(waits) > _MAX_WAITS:
                    changed = True
                    extra, keep = waits[:-_MAX_WAITS], waits[-_MAX_WAITS:]
                    for i in range(0, len(extra), _MAX_WAITS):
                        nop = bass_rust.InstNoOp(
                            name=f"waitsplit-{_wait_ctr[0]}", ins=[], outs=[])
                        _wait_ctr[0] += 1
                        nop.engine = inst.engine
                        nop.sync_info = bass_rust.SyncInfo(
                            on_wait=extra[i:i + _MAX_WAITS], on_update=[])
                        out.append(nop)
                    inst.sync_info = bass_rust.SyncInfo(
                        on_wait=keep, on_update=list(si.on_update))
                out.append(inst)
            if changed:
                blk.instructions = out


def build_program(split_waits=True):
    nc = bacc.Bacc("TRN2", num_devices=NCORES)

    # ---- I/O -------------------------------------------------------------
    xt_d = nc.dram_tensor("xt", [P, 2, N], BF16, kind="ExternalInput")
    xtloc_d = nc.dram_tensor("xtloc", [P, 2, DPC], BF16, kind="ExternalInput")
    w1p_d = nc.dram_tensor("w1p", [P, 2, 516], BF16, kind="ExternalInput")
    wad_d = nc.dram_tensor("wad", [P, 2, H], BF16, kind="ExternalInput")
    a1_d = nc.dram_tensor("a1", [P, NB, DPC], BF16, kind="ExternalInput")
    w2p_d = nc.dram_tensor("w2p", [P, 4, 66], BF16, kind="ExternalInput")
    wmu_d = nc.dram_tensor("wmu", [EMB, EMB], BF16, kind="ExternalInput")
    wlv_d = nc.dram_tensor("wlv", [EMB, EMB], BF16, kind="ExternalInput")
    b1r_d = nc.dram_tensor("b1r", [P, HID], F32, kind="ExternalInput")
    b2r_d = nc.dram_tensor("b2r", [P, EMB], F32, kind="ExternalInput")
    bmur_d = nc.dram_tensor("bmur", [P, EMB], F32, kind="ExternalInput")
    blvr_d = nc.dram_tensor("blvr", [P, EMB], F32, kind="ExternalInput")
    eps_d = nc.dram_tensor("epsl", [P, 2, EMB], F32, kind="ExternalInput")
    wdpe_d = nc.dram_tensor("wdpe", [NG_PE, P, WD_GROUP * P], F8,
                            kind="ExternalInput")
    wddve_d = nc.dram_tensor("wddve", [NG_DVE, P, 4096], F8,
                             kind="ExternalInput")
    bdpe_d = nc.dram_tensor("bdpe", [PE_ROUNDS, P, 512], BF16,
                            kind="ExternalInput")
    bddve_d = nc.dram_tensor("bddve", [NG_DVE, P, EMB], BF16,
                             kind="ExternalInput")
    outpe_d = nc.dram_tensor("outpe", [PE_ROUNDS, P, 512], F32,
                             kind="ExternalOutput")
    outdve_d = nc.dram_tensor("outdve", [NG_DVE, P, EMB], F32,
                              kind="ExternalOutput")

    # internal DRAM (broadcast round trips + collectives)
    adt_d = nc.dram_tensor("adt", [H, DPC], BF16, kind="Internal")

    with tile.TileContext(nc) as tc:
        with (
            tc.tile_pool(name="consts", bufs=1) as consts,
            tc.tile_pool(name="dram", bufs=1, space="DRAM") as dram,
            tc.tile_pool(name="sb", bufs=2) as sb,
        ):
            ident = consts.tile([P, P], F32)
            make_identity(nc, ident[:])
            ones = consts.tile([P, 1], F32)
            nc.vector.memset(ones[:], 1.0)

            # ---- const loads ---------------------------------------------
            xt_sb = consts.tile([P, 2, N], BF16)
            nc.sync.dma_start(xt_sb[:], xt_d[:])
            xtloc_sb = consts.tile([P, 2, DPC], BF16)
            nc.sync.dma_start(xtloc_sb[:], xtloc_d[:])
            w1p_sb = consts.tile([P, 2, 516], BF16)
            nc.sync.dma_start(w1p_sb[:], w1p_d[:])
            wad_sb = consts.tile([P, 2, H], BF16)
            nc.sync.dma_start(wad_sb[:], wad_d[:])
            a1_sb = consts.tile([P, NB, DPC], BF16)
            nc.sync.dma_start(a1_sb[:], a1_d[:])
            w2p_sb = consts.tile([P, 4, 66], BF16)
            nc.sync.dma_start(w2p_sb[:], w2p_d[:])
            wmu_sb = consts.tile([EMB, EMB], BF16)
            nc.sync.dma_start(wmu_sb[:], wmu_d[:])
            wlv_sb = consts.tile([EMB, EMB], BF16)
            nc.sync.dma_start(wlv_sb[:], wlv_d[:])
            b1r_sb = consts.tile([P, HID], F32)
            nc.sync.dma_start(b1r_sb[:], b1r_d[:])
            b2r_sb = consts.tile([P, EMB], F32)
            nc.sync.dma_start(b2r_sb[:], b2r_d[:])
            bmur_sb = consts.tile([P, EMB], F32)
            nc.sync.dma_start(bmur_sb[:], bmur_d[:])
            blvr_sb = consts.tile([P, EMB], F32)
            nc.sync.dma_start(blvr_sb[:], blvr_d[:])
            eps_sb = consts.tile([P, 2, EMB], F32)
            nc.sync.dma_start(eps_sb[:], eps_d[:])

            aug = consts.tile([P, NB, AUGW], BF16)
            nc.vector.memset(
                aug[:].rearrange("p b (h c) -> p b h c", h=H)[:, :, :, 0:1],
                1.0)   # ones columns only
            asrc_sb = consts.tile([P, NB, H], BF16)
            adst_rep = consts.tile([P, H, DPC], BF16)
            hidT_sb = consts.tile([P, 4, DPC], BF16)
            h2f_sb = consts.tile([P, NB, H2W], BF16)
            adst2_rep = consts.tile([P, DPC], BF16)
            embT_sb = consts.tile([EMB, 2, P], BF16)
            z32 = consts.tile([P, 2, EMB], F32)

            # ---- local a_dst1: W1ad^T @ x_loc^T, DMA-broadcast -----------
            with tc.tile_pool(name="psA", bufs=1, space="PSUM") as psA:
                padt = psA.tile([H, DPC], F32, space="PSUM")
                for ck in range(2):
                    nc.tensor.matmul(out=padt[:], lhsT=wad_sb[:, ck, :],
                                     rhs=xtloc_sb[:, ck, :],
                                     start=(ck == 0), stop=(ck == 1))
                adt_sb = sb.tile([H, DPC], BF16, tag="adt")
                nc.vector.tensor_copy(adt_sb[:], padt[:])
                nc.sync.dma_start(adt_d[:], adt_sb[:])
            for h in range(H):
                nc.sync.dma_start(
                    adst_rep[:, h, :],
                    adt_d[h:h + 1, :].to_broadcast([P, DPC]))

            # ---- phase 0: h1aug = x @ W1' --------------------------------
            hidf = sb.tile([P, 2, HID], F32, tag="hidf", bufs=1)
            rec = sb.tile([P, 2 * H], F32, tag="rec", bufs=1)
            with tc.tile_pool(name="ps0", bufs=2, space="PSUM") as ps0:
                for m in range(NB):
                    p0a = ps0.tile([P, HID], F32, space="PSUM", tag="p0a")
                    for ck in range(2):
                        nc.tensor.matmul(
                            out=p0a[:], lhsT=xt_sb[:, ck, m * P:(m + 1) * P],
                            rhs=w1p_sb[:, ck, 0:HID],
                            start=(ck == 0), stop=(ck == 1))
                    p0b = ps0.tile([P, H], F32, space="PSUM", tag="p0b")
                    for ck in range(2):
                        nc.tensor.matmul(
                            out=p0b[:], lhsT=xt_sb[:, ck, m * P:(m + 1) * P],
                            rhs=w1p_sb[:, ck, HID:HID + H],
                            start=(ck == 0), stop=(ck == 1))
                    nc.scalar.copy(
                        aug[:, m, 0:516].rearrange(
                            "p (h c) -> p h c", h=H)[:, :, 1:129],
                        p0a[:].rearrange("p (h c) -> p h c", h=H))
                    nc.scalar.copy(asrc_sb[:, m, :], p0b[:])

                # ---- layer-1 dense edge pass, head-major (one open
                # accumulation group pair per head; a psum bank cannot host
                # two concurrent groups: start pending-zeroes the full bank)
                with tc.tile_pool(name="ps1", bufs=2, space="PSUM") as ps1:
                    for h in range(H):
                        pdh = [ps1.tile([P, 129], F32, space="PSUM",
                                        tag=f"pd{half}", name=f"pd{half}")
                               for half in range(2)]
                        for m0 in range(0, NB, 4):
                            lg = sb.tile([P, 4, DPC], BF16, tag="lg")
                            nc.vector.tensor_tensor(
                                out=lg[:],
                                in0=adst_rep[:, h, :][:, None, :]
                                    .to_broadcast([P, 4, DPC]),
                                in1=asrc_sb[:, m0:m0 + 4, h:h + 1]
                                    .to_broadcast([P, 4, DPC]),
                                op=OP.add)
                            lk = sb.tile([P, 4, DPC], BF16, tag="lk")
                            nc.vector.scalar_tensor_tensor(
                                out=lk[:], in0=lg[:], scalar=NEG, in1=lg[:],
                                op0=OP.mult, op1=OP.max)
                            ev = sb.tile([P, 4, DPC], BF16, tag="ev")
                            nc.scalar.activation(ev[:], lk[:], AF.Exp)
                            mt = sb.tile([P, 4, DPC], BF16, tag="mt")
                            nc.vector.tensor_tensor(
                                out=mt[:], in0=ev[:],
                                in1=a1_sb[:, m0:m0 + 4, :], op=OP.mult)
                            for mi in range(4):
                                m = m0 + mi
                                for half in range(2):
                                    nc.tensor.matmul(
                                        out=pdh[half][:],
                                        lhsT=mt[:, mi,
                                                half * P:(half + 1) * P],
                                        rhs=aug[:, m, h * 129:(h + 1) * 129],
                                        start=(m == 0), stop=(m == NB - 1))
                        for half in range(2):
                            nc.vector.tensor_copy(
                                rec[:, h * 2 + half:h * 2 + half + 1],
                                pdh[half][:, 0:1])
                            nc.vector.reciprocal(
                                rec[:, h * 2 + half:h * 2 + half + 1],
                                rec[:, h * 2 + half:h * 2 + half + 1])
                            nc.vector.scalar_tensor_tensor(
                                out=hidf[:, half, h * P:(h + 1) * P],
                                in0=pdh[half][:, 1:129],
                                scalar=rec[:, h * 2 + half:h * 2 + half + 1],
                                in1=b1r_sb[:, h * P:(h + 1) * P],
                                op0=OP.mult, op1=OP.add)
            for half in range(2):
                nc.scalar.activation(hidf[:, half, :], hidf[:, half, :],
                                     AF.Relu)

            # ---- transpose hidden, local h2aug, AllGather ----------------
            h2loc = dram.tile([DPC, H2W], BF16)
            h2full = dram.tile([N, H2W], BF16)
            with tc.tile_pool(name="psT", bufs=2, space="PSUM") as psT:
                for half in range(2):
                    for ck in range(4):
                        pt = psT.tile([P, P], F32, space="PSUM", tag="pt")
                        nc.tensor.transpose(
                            out=pt[:], in_=hidf[:, half, ck * P:(ck + 1) * P],
                            identity=ident[:])
                        nc.vector.tensor_copy(
                            hidT_sb[:, ck, half * P:(half + 1) * P], pt[:])
            with (
                tc.tile_pool(name="ps2a", bufs=1, space="PSUM") as ps2a,
                tc.tile_pool(name="ps2t", bufs=2, space="PSUM") as ps2t,
            ):
                ph2t = ps2a.tile([66, DPC], F32, space="PSUM")
                for ck in range(4):
                    nc.tensor.matmul(out=ph2t[:], lhsT=w2p_sb[:, ck, :],
                                     rhs=hidT_sb[:, ck, :],
                                     start=(ck == 0), stop=(ck == 3))
                h2at = sb.tile([66, DPC], F32, tag="h2at")
                nc.vector.tensor_copy(h2at[:], ph2t[:])
                h2l_sb = sb.tile([P, 2, H2W], BF16, tag="h2l")
                nc.vector.memset(h2l_sb[:], 1.0)
                for half in range(2):
                    pt2 = ps2t.tile([P, 66], F32, space="PSUM", tag="pt2")
                    nc.tensor.transpose(
                        out=pt2[:], in_=h2at[:, half * P:(half + 1) * P],
                        identity=ident[0:66, 0:66])
                    nc.scalar.copy(h2l_sb[:, half, 1:H2W], pt2[:])
                for half in range(2):
                    nc.sync.dma_start(h2loc[half * P:(half + 1) * P, :],
                                      h2l_sb[:, half, :])
            nc.gpsimd.collective_compute(
                "AllGather", OP.bypass, replica_groups=RG,
                ins=[h2loc.opt()], outs=[h2full.opt()])
            nc.sync.dma_start(
                h2f_sb[:],
                h2full[:, :].rearrange("(b p) f -> p b f", p=P))
            nc.sync.dma_start(
                adst2_rep[:],
                h2loc[:, 66:67].rearrange("a b -> b a").to_broadcast(
                    [P, DPC]))


            # ---- layer-2 dense edge pass ---------------------------------
            zs_in = dram.tile([EMB, 1], F32)
            zs_out = dram.tile([EMB, 1], F32)
            with tc.tile_pool(name="ps2", bufs=1, space="PSUM") as ps2:
                pe2 = [ps2.tile([P, 66], F32, space="PSUM", tag=f"pe2{half}",
                                name=f"pe2{half}") for half in range(2)]
                for m0 in range(0, NB, 16):
                    lg2 = sb.tile([P, NB, DPC], BF16, tag="lg2", bufs=1)
                    nc.vector.tensor_tensor(
                        out=lg2[:],
                        in0=adst2_rep[:][:, None, :].to_broadcast(
                            [P, NB, DPC]),
                        in1=h2f_sb[:, m0:m0 + 16, 65:66].to_broadcast(
                            [P, NB, DPC]),
                        op=OP.add)
                    lk2 = sb.tile([P, NB, DPC], BF16, tag="lk2", bufs=1)
                    nc.vector.scalar_tensor_tensor(
                        out=lk2[:], in0=lg2[:], scalar=NEG, in1=lg2[:],
                        op0=OP.mult, op1=OP.max)
                    ev2 = sb.tile([P, NB, DPC], BF16, tag="ev2", bufs=1)
                    nc.scalar.activation(ev2[:], lk2[:], AF.Exp)
                    m2 = sb.tile([P, NB, DPC], BF16, tag="m2", bufs=1)
                    nc.vector.tensor_tensor(
                        out=m2[:], in0=ev2[:], in1=a1_sb[:, m0:m0 + 16, :],
                        op=OP.mult)
                    for mi in range(16):
                        m = m0 + mi
                        for half in range(2):
                            nc.tensor.matmul(
                                out=pe2[half][:, 0:65],
                                lhsT=m2[:, mi, half * P:(half + 1) * P],
                                rhs=h2f_sb[:, m, 0:65],
                                start=(m == 0), stop=(m == NB - 1))

                rec2 = sb.tile([P, 2], F32, tag="rec2")
                for half in range(2):
                    nc.vector.tensor_copy(rec2[:, half:half + 1],
                                          pe2[half][:, 0:1])
                nc.vector.reciprocal(rec2[:], rec2[:])
                emb32 = sb.tile([P, 2, EMB], F32, tag="emb32", bufs=1)
                for half in range(2):
                    nc.vector.scalar_tensor_tensor(
                        out=emb32[:, half, :], in0=pe2[half][:, 1:65],
                        scalar=rec2[:, half:half + 1], in1=b2r_sb[:],
                        op0=OP.mult, op1=OP.add)

            # ---- mu / logvar / z / z-sum ---------------------------------
            with tc.tile_pool(name="ps3", bufs=1, space="PSUM") as ps3:
                pzs = ps3.tile([EMB, 1], F32, space="PSUM", tag="pzs")
                for half in range(2):
                    pt3 = ps3.tile([EMB, P], F32, space="PSUM", tag="pt3",
                                   bufs=2)
                    nc.tensor.transpose(out=pt3[:], in_=emb32[:, half, :],
                                        identity=ident[:])
                    nc.vector.tensor_copy(embT_sb[:, half, :], pt3[:])
                for half in range(2):
                    pmu = ps3.tile([P, EMB], F32, space="PSUM", tag="pmu")
                    nc.tensor.matmul(out=pmu[:], lhsT=embT_sb[:, half, :],
                                     rhs=wmu_sb[:], start=True, stop=True)
                    plv = ps3.tile([P, EMB], F32, space="PSUM", tag="plv")
                    nc.tensor.matmul(out=plv[:], lhsT=embT_sb[:, half, :],
                                     rhs=wlv_sb[:], start=True, stop=True)
                    elv = sb.tile([P, EMB], F32, tag="elv")
                    nc.vector.tensor_add(elv[:], plv[:], blvr_sb[:])
                    nc.scalar.activation(elv[:], elv[:], AF.Exp, scale=0.5)
                    nc.vector.tensor_tensor(out=elv[:], in0=elv[:],
                                            in1=eps_sb[:, half, :],
                                            op=OP.mult)
                    nc.vector.tensor_add(elv[:], elv[:], bmur_sb[:])
                    nc.vector.tensor_add(z32[:, half, :], elv[:], pmu[:])
                for half in range(2):
                    nc.tensor.matmul(out=pzs[:], lhsT=z32[:, half, :],
                                     rhs=ones[:], start=(half == 0),
                                     stop=(half == 1))
                zsum_sb = sb.tile([EMB, 1], F32, tag="zsum")
                nc.vector.tensor_copy(zsum_sb[:], pzs[:])
                nc.sync.dma_start(zs_in[:], zsum_sb[:])

            nc.gpsimd.collective_compute(
                "AllReduce", OP.add, replica_groups=RG,
                ins=[zs_in.opt()], outs=[zs_out.opt()])

            # ---- decoder -------------------------------------------------
            rhs_zm = consts.tile([P, 2], F32)
            nc.vector.memset(rhs_zm[:], 0.0)
            nc.sync.dma_start(rhs_zm[0:EMB, 0:1], zs_out[:])
            nc.sync.dma_start(rhs_zm[EMB:2 * EMB, 1:2], zs_out[:])
            rhs_zmq = consts.tile([P, 2], F8)
            nc.vector.tensor_scalar(out=rhs_zmq[:], in0=rhs_zm[:],
                                    scalar1=SZ, scalar2=None, op0=OP.mult)
            zmr32 = consts.tile([P, EMB], F32)
            nc.sync.dma_start(
                zmr32[:],
                zs_out[:, :].rearrange("a b -> b a").to_broadcast([P, EMB]))
            zm_repb = consts.tile([P, 32, EMB], BF16)
            nc.vector.tensor_copy(
                zm_repb[:],
                zmr32[:][:, None, :].to_broadcast([P, 32, EMB]))

            with (
                tc.tile_pool(name="wd", bufs=1) as wdp,
                tc.tile_pool(name="dec", bufs=2) as decp,
                tc.tile_pool(name="dv", bufs=2) as dvp,
                tc.tile_pool(name="ps4", bufs=2, space="PSUM") as ps4,
            ):
                pdec = None
                for g in range(NG_PE):
                    wd_sb = wdp.tile([P, WD_GROUP * P], F8, tag="wd",
                                     bufs=WPE_BUFS)
                    nc.scalar.dma_start(wd_sb[:], wdpe_d[g, :, :])
                    if g % 8 == 0:
                        pdec = ps4.tile([P, 512], F32, space="PSUM",
                                        tag="pdec")
                    for u in range(WD_GROUP):
                        t = g * WD_GROUP + u
                        u2 = t % 256
                        nc.tensor.matmul(
                            out=pdec[:, 2 * u2:2 * u2 + 2],
                            lhsT=wd_sb[:, u * P:(u + 1) * P], rhs=rhs_zmq[:],
                            start=True, stop=True)
                    if g % 8 == 7 or g == NG_PE - 1:
                        b = g // 8
                        w = 512 if g % 8 == 7 else (g % 8 + 1) * 64
                        bd_sb = decp.tile([P, 512], BF16, tag="bd")
                        nc.scalar.dma_start(bd_sb[:, 0:w], bdpe_d[b, :, 0:w])
                        so = decp.tile([P, 512], F32, tag="so")
                        nc.vector.scalar_tensor_tensor(
                            out=so[:, 0:w], in0=pdec[:, 0:w], scalar=DESC_PE,
                            in1=bd_sb[:, 0:w], op0=OP.mult, op1=OP.add)
                        nc.scalar.activation(so[:, 0:w], so[:, 0:w],
                                             AF.Sigmoid)
                        nc.sync.dma_start(outpe_d[b, :, 0:w], so[:, 0:w])

                for gg in range(NG_DVE):
                    wdt_sb = wdp.tile([P, 4096], F8, tag="wdt",
                                      bufs=WDVE_BUFS)
                    nc.scalar.dma_start(wdt_sb[:], wddve_d[gg, :, :])
                    bdt_sb = decp.tile([P, EMB], BF16, tag="bdt")
                    nc.scalar.dma_start(bdt_sb[:], bddve_d[gg, :, :])
                    lo = dvp.tile([P, EMB], F32, tag="lo")
                    for hh in range(2):
                        prod = dvp.tile([P, 32, EMB], BF16, tag="prod")
                        nc.vector.tensor_tensor(
                            out=prod[:],
                            in0=wdt_sb[:, hh * 2048:(hh + 1) * 2048]
                                .rearrange("p (c k) -> p c k", k=EMB),
                            in1=zm_repb[:], op=OP.mult)
                        nc.vector.tensor_reduce(
                            out=lo[:, hh * 32:(hh + 1) * 32], in_=prod[:],
                            axis=mybir.AxisListType.X, op=OP.add)
                    so2 = dvp.tile([P, EMB], F32, tag="so2")
                    nc.vector.scalar_tensor_tensor(
                        out=so2[:], in0=lo[:], scalar=DESC_DVE,
                        in1=bdt_sb[:], op0=OP.mult, op1=OP.add)
                    nc.scalar.activation(so2[:], so2[:], AF.Sigmoid)
                    nc.sync.dma_start(outdve_d[gg, :, :], so2[:])


    nc.compile()
    if split_waits:
        _split_excess_waits(nc)
    return nc


_prog_cache = {}


def _get_program():
    if 0 not in _prog_cache:
        _prog_cache[0] = build_program()
    return _prog_cache[0]


def prepare_inputs(inputs):
    bf = ml_dtypes.bfloat16
    f8 = ml_dtypes.float8_e4m3fn
    edge_index = np.asarray(inputs["edge_index"])
    x = np.asarray(inputs["x"], np.float32)
    eps = np.asarray(inputs["eps"], np.float32)
    W1 = np.asarray(inputs["W1"], np.float32)
    as1 = np.asarray(inputs["att_src1"], np.float32)
    ad1 = np.asarray(inputs["att_dst1"], np.float32)
    W2 = np.asarray(inputs["W2"], np.float32)
    as2 = np.asarray(inputs["att_src2"], np.float32).ravel()
    ad2 = np.asarray(inputs["att_dst2"], np.float32).ravel()
    Wmu = np.asarray(inputs["Wmu"], np.float32)
    Wlv = np.asarray(inputs["Wlv"], np.float32)
    Wd = np.asarray(inputs["Wd"], np.float32)
    bd = np.asarray(inputs["bd"], np.float32)

    # dense multiplicity matrix with self loops
    loops = np.arange(N, dtype=np.int64)
    src = np.concatenate([edge_index[0].astype(np.int64), loops])
    dst = np.concatenate([edge_index[1].astype(np.int64), loops])
    A = np.zeros((N, N), np.float32)
    np.add.at(A, (src, dst), 1.0)

    # fold attention dots into layer weights
    Was = (W1.reshape(F_IN, H, C1) * as1).sum(-1)           # [256, H]
    Wad = (W1.reshape(F_IN, H, C1) * ad1).sum(-1)           # [256, H]
    W1p = np.concatenate([W1, Was], axis=1)                 # [256, 516]
    W2p = np.concatenate([W2, (W2 * as2).sum(1)[:, None],
                          (W2 * ad2).sum(1)[:, None]], axis=1)  # [512, 66]

    xT = np.ascontiguousarray(x.T).astype(bf)               # [256, 2048]
    common = {
        "xt": np.ascontiguousarray(
            xT.reshape(2, P, N).transpose(1, 0, 2)),
        "w1p": np.ascontiguousarray(
            W1p.astype(bf).reshape(2, P, 516).transpose(1, 0, 2)),
        "wad": np.ascontiguousarray(
            Wad.astype(bf).reshape(2, P, H).transpose(1, 0, 2)),
        "w2p": np.ascontiguousarray(
            W2p.astype(bf).reshape(4, P, 66).transpose(1, 0, 2)),
        "wmu": Wmu.astype(bf),
        "wlv": Wlv.astype(bf),
        "b1r": np.tile(np.asarray(inputs["b1"], np.float32)[None, :],
                       (P, 1)),
        "b2r": np.tile(np.asarray(inputs["b2"], np.float32)[None, :],
                       (P, 1)),
        "bmur": np.tile(np.asarray(inputs["bmu"], np.float32)[None, :],
                        (P, 1)),
        "blvr": np.tile(np.asarray(inputs["blv"], np.float32)[None, :],
                        (P, 1)),
    }

    Wdq = np.clip(Wd * SW, -240.0, 240.0)
    in_maps = []
    for c in range(NCORES):
        m = dict(common)
        m["xtloc"] = np.ascontiguousarray(
            xT[:, c * DPC:(c + 1) * DPC].reshape(2, P, DPC)
            .transpose(1, 0, 2))
        m["a1"] = np.ascontiguousarray(
            A[:, c * DPC:(c + 1) * DPC].reshape(NB, P, DPC)
            .transpose(1, 0, 2).astype(bf))
        m["epsl"] = np.ascontiguousarray(
            eps[c * DPC:(c + 1) * DPC].reshape(2, P, EMB)
            .transpose(1, 0, 2))

        base = c * COLS
        wpe = Wdq[:, base:base + PE_COLS]                   # [64, 327680]
        X = wpe.reshape(EMB, NG_PE * WD_GROUP, 2, P)
        lhsT = np.zeros((NG_PE * WD_GROUP, P, P), np.float32)
        lhsT[:, 0:EMB, :] = X[:, :, 0, :].transpose(1, 0, 2)
        lhsT[:, EMB:P, :] = X[:, :, 1, :].transpose(1, 0, 2)
        m["wdpe"] = np.ascontiguousarray(
            lhsT.reshape(NG_PE, WD_GROUP, P, P)
                .transpose(0, 2, 1, 3).reshape(NG_PE, P, WD_GROUP * P)
                .astype(f8))
        wdv = Wdq[:, base + PE_COLS:base + COLS]
        m["wddve"] = np.ascontiguousarray(
            wdv.reshape(EMB, NG_DVE, EMB, P).transpose(1, 3, 2, 0)
               .reshape(NG_DVE, P, 4096).astype(f8))
        bpe = np.zeros(PE_ROUNDS * 65536, np.float32)
        bpe[:PE_COLS] = bd[base:base + PE_COLS]
        bpe = bpe.reshape(PE_ROUNDS, 256, 2, P)
        m["bdpe"] = np.ascontiguousarray(
            bpe.transpose(0, 3, 1, 2).reshape(PE_ROUNDS, P, 512).astype(bf))
        bdv = bd[base + PE_COLS:base + COLS]
        m["bddve"] = np.ascontiguousarray(
            bdv.reshape(NG_DVE, EMB, P).transpose(0, 2, 1).astype(bf))
        in_maps.append(m)
    return in_maps


def assemble_output(results):
    decoded = np.empty((N, N), np.float32)
    for c in range(NCORES):
        ope = results[c]["outpe"]
        fpe = ope.reshape(PE_ROUNDS, P, 256, 2).transpose(0, 2, 3, 1) \
                 .reshape(PE_ROUNDS * 65536)[:PE_COLS]
        odv = results[c]["outdve"]
        fdv = odv.transpose(0, 2, 1).reshape(DVE_COLS)
        decoded[c * DPC:(c + 1) * DPC, :] = np.concatenate(
            [fpe, fdv]).reshape(DPC, N)
    return decoded


def run(inputs, **run_kwargs):
    in_maps = prepare_inputs(inputs)
    nc = _get_program()
    last_err = None
    for _attempt in range(3):
        try:
            res = run_bass_kernel_spmd(nc, in_maps,
                                       core_ids=list(range(NCORES)),
                                       **run_kwargs)
            return assemble_output(res.results), res
        except Exception as e:  # transient NRT device errors
            last_err = e
    raise last_err


def kernel(**inputs):
    out, _ = run(inputs)
    return out

